# revision 26
# baseline (speedup 1.0000x reference)
"""Trainium2 Bass kernel for nn_DeformableNCC (B=64,K=32,T=1280,C=16,E=64).

Math: the reference's per-(b,k) linear-interp warp of prototypes reduces exactly to
    out[b,k,t,:] = S0[k,t] + relu(w)*Up[k,t] + relu(-w)*Un[k,t] + off[b,k]
with Up[k,t] = Pext[k,t-1]-P[k,t], Un[k,t] = Pext[k,t+1]-P[k,t] (edge-replicated,
boundary rows zeroed), valid whenever |warp| < 1 (actual range ~[-0.36, 0.40]).
The warp/off scalars come from the small conv encoder, computed on-device.

Sharding: pure data-parallel over batch (8 rows per core, 8 cores).
Per-core layout: prototype tap tiles are stored bf16 as [128, 4*1280] with
partition p = g*16 + th, free f = q*1280 + tl*16 + c, where k = 8*q + g and
t = th*80 + tl.  This makes every per-(b,q) pass a single [128, 1280] op with
per-partition scalars (each partition group of 16 = one k), and the output DMA
a fully contiguous 640KB store.
"""
import sys

for p in ("/opt/trn_rl_repo",):
    if p not in sys.path:
        sys.path.insert(0, p)

import numpy as np
import ml_dtypes

import concourse.bass as bass
import concourse.mybir as mybir
from concourse import tile, bacc, bass_utils

BF16 = ml_dtypes.bfloat16
B, K, T, C, E = 64, 32, 1280, 16, 64
CONV_CH, POOL = 32, 10
NCORES = 8
BL = B // NCORES        # 8 batch rows per core
NQ = K // 8             # 4 k-groups
TH, TL = 4, 320         # t = th*320 + tl ; partition p = k*4 + th
FD = TL * C             # 5120 free elems per (k, th) row; quarters of 1280
QF = FD // 4

_f32 = mybir.dt.float32
_bf16 = mybir.dt.bfloat16
_i32 = mybir.dt.int32
AF = mybir.ActivationFunctionType
OP = mybir.AluOpType


def _tap_layout(a):
    """[K, T, C] -> [128, FD]: partition = (k, th), free = (tl, c) — a reshape."""
    return np.ascontiguousarray(a.reshape(128, FD))


def build_nc():
    nc = bacc.Bacc(None, target_bir_lowering=False)

    # ---- DRAM I/O ----
    s0_d = nc.dram_tensor("s0", [128, FD], _bf16, kind="ExternalInput")
    up_d = nc.dram_tensor("up", [128, FD], _bf16, kind="ExternalInput")
    un_d = nc.dram_tensor("un", [128, FD], _bf16, kind="ExternalInput")
    xt_d = nc.dram_tensor("xt", [80, BL * T], _bf16, kind="ExternalInput")
    cw_d = nc.dram_tensor("cw", [80, CONV_CH], _bf16, kind="ExternalInput")
    cb_d = nc.dram_tensor("cb", [CONV_CH, 1], _f32, kind="ExternalInput")
    lw_d = nc.dram_tensor("lw", [96, POOL * E], _f32, kind="ExternalInput")
    cb4_d = nc.dram_tensor("cb4", [96, 1], _f32, kind="ExternalInput")
    lb_d = nc.dram_tensor("lb", [E, 1], _f32, kind="ExternalInput")
    ww_d = nc.dram_tensor("ww", [E, K], _f32, kind="ExternalInput")
    wb_d = nc.dram_tensor("wb", [K, 1], _f32, kind="ExternalInput")
    ow_d = nc.dram_tensor("ow", [E, K], _f32, kind="ExternalInput")
    ob_d = nc.dram_tensor("ob", [K, 1], _f32, kind="ExternalInput")

    out_d = nc.dram_tensor("out", [BL, K, T, C], _f32, kind="ExternalOutput")
    warp_d = nc.dram_tensor("warp_o", [BL, K], _f32, kind="ExternalOutput")
    off_d = nc.dram_tensor("off_o", [BL, K], _f32, kind="ExternalOutput")

    with tile.TileContext(nc) as tc:
        with (
            tc.tile_pool(name="taps", bufs=1) as taps,
            tc.tile_pool(name="enc", bufs=1) as enc,
            tc.tile_pool(name="wk", bufs=2) as wk,
            tc.tile_pool(name="outp", bufs=2) as outp,
        ):
            enc_psum = tc.tile_pool(name="pconv", bufs=2, space=bass.MemorySpace.PSUM)
            pconv = enc_psum.__enter__()
            enc_psum2 = tc.tile_pool(name="psmall", bufs=2, space=bass.MemorySpace.PSUM)
            psmall = enc_psum2.__enter__()
            # ---- ACT warmup: force the lazy ACT_TABLE_LOAD at t~0 ----
            wrm = enc.tile([1, 2], _f32)
            nc.gpsimd.memset(wrm[:], 0.0)
            nc.scalar.activation(wrm[:], wrm[:], AF.Relu, bias=0.0, scale=1.0)

            # ---- tiny weights first, then im2col, then taps (FIFO queue) ----
            cw_t = enc.tile([80, CONV_CH], _bf16)
            cb_t = enc.tile([CONV_CH, 1], _f32)
            nc.scalar.dma_start(cw_t[:], cw_d[:])
            nc.scalar.dma_start(cb_t[:], cb_d[:])
            cb3_t = enc.tile([96, 1], _f32)
            nc.scalar.dma_start(cb3_t[:], cb4_d[:])
            lw_t = enc.tile([96, POOL * E], _f32)
            lb_t = enc.tile([E, 1], _f32)
            nc.scalar.dma_start(lw_t[:], lw_d[:])
            nc.scalar.dma_start(lb_t[:], lb_d[:])
            ww_t = enc.tile([E, K], _f32)
            wb_t = enc.tile([K, 1], _f32)
            ow_t = enc.tile([E, K], _f32)
            obb_t = enc.tile([K, 1], _f32)
            nc.scalar.dma_start(ww_t[:], ww_d[:])
            nc.scalar.dma_start(wb_t[:], wb_d[:])
            nc.scalar.dma_start(ow_t[:], ow_d[:])
            nc.scalar.dma_start(obb_t[:], ob_d[:])

            # ---- im2col: one contiguous 80-partition load (host-built) ----
            ic = enc.tile([80, BL * T], _bf16)          # [d*16+c, b*1280+t]
            nc.sync.dma_start(ic[:], xt_d[:])

            # ---- taps last: up/un first (t1/t2 need them before s0) ----
            s0_t = taps.tile([128, FD], _bf16)
            up_t = taps.tile([128, FD], _bf16)
            un_t = taps.tile([128, FD], _bf16)
            nc.sync.dma_start(up_t[:], up_d[:])
            nc.sync.dma_start(un_t[:], un_d[:])
            nc.sync.dma_start(s0_t[:], s0_d[:])

            # ---- conv: b=3m+n -> partition group 32m, free n*1280 ----
            # (PE matmul out base partition must be 0/32/64 -> 3 groups)
            h3 = enc.tile([96, 3 * T], _bf16)
            pf3 = enc.tile([96, 3 * POOL], _f32)
            for n in range(3):
                nm = 3 if n < 2 else 2           # m-count in this n-column
                ps = pconv.tile([96, T], _f32, tag="ps")
                for m in range(nm):
                    b = 3 * m + n
                    for c0, c1 in ((0, 512), (512, 1024), (1024, T)):
                        nc.tensor.matmul(ps[32 * m:32 * (m + 1), c0:c1], cw_t[:],
                                         ic[:, b * T + c0:b * T + c1],
                                         start=True, stop=True)
                nc.scalar.activation(h3[0:32 * nm, bass.ts(n, T)], ps[0:32 * nm, :],
                                     AF.Relu, bias=cb3_t[0:32 * nm, :], scale=1.0)
                # pool this n-group immediately (overlaps next conv group)
                nc.vector.tensor_reduce(
                    pf3[0:32 * nm, bass.ts(n, POOL)],
                    h3[0:32 * nm, bass.ts(n, T)].rearrange(
                        "p (g i) -> p g i", i=T // POOL),
                    mybir.AxisListType.X, OP.add)

            # ---- linear + relu: per-(m,w) matmuls overlap the reduces ----
            pse = psmall.tile([E, BL], _f32, tag="small")
            pfv = pf3[:].rearrange("p (n w) -> p w n", w=POOL)
            for m in range(3):
                ncols = 3 if m < 2 else 2
                for w in range(POOL):
                    nc.tensor.matmul(pse[:, 3 * m:3 * m + ncols],
                                     lw_t[32 * m:32 * (m + 1), bass.ts(w, E)],
                                     pfv[32 * m:32 * (m + 1), w, 0:ncols],
                                     start=(m == 0 and w == 0),
                                     stop=(m == 2 and w == POOL - 1))
            emb = enc.tile([E, BL], _f32)
            nc.scalar.activation(emb[:], pse[:], AF.Relu,
                                 bias=lb_t[:], scale=1.0 / (T // POOL))
            # ---- heads ----
            psw = psmall.tile([K, BL], _f32, tag="small")
            nc.tensor.matmul(psw[:], ww_t[:], emb[:], start=True, stop=True)
            warp_sb = enc.tile([K, BL], _f32)
            nc.vector.tensor_scalar(warp_sb[:], psw[:], wb_t[:], None, OP.add)

            pso = psmall.tile([K, BL], _f32, tag="small")
            nc.tensor.matmul(pso[:], ow_t[:], emb[:], start=True, stop=True)
            off_sb = enc.tile([K, BL], _f32)
            nc.vector.tensor_scalar(off_sb[:], pso[:], obb_t[:], None, OP.add)

            with nc.allow_non_contiguous_dma(reason="tiny [K,BL] transposed store"):
                nc.gpsimd.dma_start(warp_d[:].rearrange("b k -> k b"), warp_sb[:])
                nc.gpsimd.dma_start(off_d[:].rearrange("b k -> k b"), off_sb[:])

            # ---- per-pair scalars: [wp | wn | off] as [K, 3*BL] ----
            scal = enc.tile([K, 3 * BL], _f32)
            nc.vector.tensor_scalar(scal[:, 0:BL], warp_sb[:], 0.0, None, OP.max)
            nc.vector.tensor_scalar(scal[:, BL:2 * BL], warp_sb[:], -1.0, 0.0,
                                    OP.mult, OP.max)
            nc.vector.tensor_copy(scal[:, 2 * BL:3 * BL], off_sb[:])

            # ---- replicate to [128, 3*BL] via one-hot matmul (p//4 == k) ----
            e2 = enc.tile([K, 128], _i32)
            nc.gpsimd.iota(e2[:], pattern=[[1, 32], [0, 4]], base=0,
                           channel_multiplier=-1)
            rq = enc.tile([K, 128], _f32)
            nc.vector.tensor_scalar(rq[:], e2[:], 0, None, OP.is_equal)
            pss = psmall.tile([128, 3 * BL], _f32, tag="small")
            nc.tensor.matmul(pss[:], rq[:], scal[:], start=True, stop=True)
            scalq = enc.tile([128, 3 * BL], _f32)
            nc.vector.tensor_copy(scalq[:], pss[:])

            enc_psum2.__exit__(None, None, None)
            enc_psum.__exit__(None, None, None)
            pmain_cm = tc.tile_pool(name="pmain", bufs=2, space=bass.MemorySpace.PSUM)
            pmain = pmain_cm.__enter__()

            # ---- bf16 identity for PE tap-joins ----
            ei = enc.tile([128, 128], _i32)
            nc.gpsimd.iota(ei[:], pattern=[[-1, 128]], base=0,
                           channel_multiplier=1)          # q - p
            ident = taps.tile([128, 128], _bf16)
            nc.vector.tensor_scalar(ident[:], ei[:], 0, None, OP.is_equal)

            # ---- main: per-b [128, 5120] DVE; quarter psum+evac; half DMAs ----
            for b in range(BL):
                wp = scalq[:, b:b + 1]
                wn = scalq[:, BL + b:BL + b + 1]
                of = scalq[:, 2 * BL + b:2 * BL + b + 1]

                t1 = wk.tile([128, FD], _bf16, tag="t1")
                nc.vector.tensor_scalar(t1[:], up_t[:], wp, None, OP.mult)
                t2 = wk.tile([128, FD], _bf16, tag="t2")
                nc.vector.tensor_scalar(t2[:], un_t[:], wn, None, OP.mult)
                u = wk.tile([128, FD], _bf16, tag="u")

                ofb = outp.tile([128, FD], _f32, tag="ofb")
                for q4 in range(4):
                    qs = bass.ts(q4, QF)
                    nc.vector.tensor_tensor(u[:, qs], t1[:, qs], t2[:, qs],
                                            OP.add)
                    ps = pmain.tile([128, QF], _f32, tag="ps")
                    for c0, c1 in ((0, 512), (512, 1024), (1024, QF)):
                        s = q4 * QF
                        nc.tensor.matmul(ps[:, c0:c1], ident[:],
                                         u[:, s + c0:s + c1],
                                         start=True, stop=False)
                        nc.tensor.matmul(ps[:, c0:c1], ident[:],
                                         s0_t[:, s + c0:s + c1],
                                         start=False, stop=True)
                    nc.scalar.activation(ofb[:, bass.ts(q4, QF)], ps[:],
                                         AF.Identity, bias=of, scale=1.0)
                    if q4 % 2 == 1:
                        h = q4 // 2
                        dst = out_d[b].rearrange(
                            "k (th tl) c -> (k th) (tl c)", th=TH)
                        nc.sync.dma_start(
                            dst[:, bass.ts(h, FD // 2)],
                            ofb[:, bass.ts(h, FD // 2)])
            pmain_cm.__exit__(None, None, None)

    nc.compile()
    return nc


def host_prep(inputs):
    """Build the per-core in_maps from the full reference inputs (numpy only)."""
    x = np.asarray(inputs["x"], np.float32)
    P = np.asarray(inputs["prototypes"], np.float32)
    conv_w = np.asarray(inputs["conv_w"], np.float32)
    conv_b = np.asarray(inputs["conv_b"], np.float32)
    lin_w = np.asarray(inputs["lin_w"], np.float32)
    lin_b = np.asarray(inputs["lin_b"], np.float32)
    warp_W = np.asarray(inputs["warp_W"], np.float32)
    warp_b = np.asarray(inputs["warp_b"], np.float32)
    off_W = np.asarray(inputs["off_W"], np.float32)
    off_b = np.asarray(inputs["off_b"], np.float32)

    # taps (weight-only prep): S0, Up, Un in tap layout, bf16
    Pm1 = np.concatenate([P[:, :1], P[:, :-1]], axis=1)
    Pp1 = np.concatenate([P[:, 1:], P[:, -1:]], axis=1)
    s0 = _tap_layout(P.astype(BF16))
    up = _tap_layout((Pm1 - P).astype(BF16))
    un = _tap_layout((Pp1 - P).astype(BF16))

    cw = np.ascontiguousarray(
        conv_w.transpose(2, 1, 0).reshape(80, CONV_CH)).astype(BF16)  # [d*16+c, o]
    cb = conv_b.reshape(CONV_CH, 1).copy()
    lw1 = np.ascontiguousarray(
        lin_w.reshape(E, CONV_CH, POOL).transpose(1, 2, 0).reshape(
            CONV_CH, POOL * E))  # [o, win*64+e]
    lw = np.tile(lw1, (3, 1))
    cb4 = np.tile(conv_b.reshape(CONV_CH, 1), (3, 1))
    lb = lin_b.reshape(E, 1).copy()
    ww = np.ascontiguousarray(warp_W.T)
    wb = warp_b.reshape(K, 1).copy()
    ow = np.ascontiguousarray(off_W.T)
    ob = off_b.reshape(K, 1).copy()

    shared = dict(s0=s0, up=up, un=un, cw=cw, cb=cb, cb4=cb4, lw=lw, lb=lb,
                  ww=ww, wb=wb, ow=ow, ob=ob)
    xt = np.ascontiguousarray(x.transpose(0, 2, 1)).astype(BF16)  # [B, C, T]
    xt_pad = np.zeros((B, C, T + 4), BF16)
    xt_pad[:, :, 2:T + 2] = xt
    in_maps = []
    for i in range(NCORES):
        m = dict(shared)
        xtp = xt_pad[i * BL:(i + 1) * BL]              # [BL, C, T+4]
        icol = np.stack([xtp[:, :, d:d + T] for d in range(5)], axis=0)
        m["xt"] = np.ascontiguousarray(
            icol.transpose(0, 2, 1, 3).reshape(80, BL * T))  # [d*16+c, b*t]
        in_maps.append(m)
    return in_maps


_NC_CACHE = {}


def kernel(**inputs):
    if "nc" not in _NC_CACHE:
        _NC_CACHE["nc"] = build_nc()
    nc = _NC_CACHE["nc"]
    in_maps = host_prep(inputs)
    res = bass_utils.run_bass_kernel_spmd(nc, in_maps, core_ids=list(range(NCORES)))
    outs = res.results
    warped = np.concatenate([np.asarray(o["out"]) for o in outs], axis=0)
    warp = np.concatenate([np.asarray(o["warp_o"]) for o in outs], axis=0)
    off = np.concatenate([np.asarray(o["off_o"]) for o in outs], axis=0)
    return warped, warp[..., None], off[..., None]


# revision 27
# speedup vs baseline: 1.1075x; 1.1075x over previous
"""Trainium2 Bass kernel for nn_DeformableNCC (B=64,K=32,T=1280,C=16,E=64).

Math: the reference's per-(b,k) linear-interp warp of prototypes reduces exactly to
    out[b,k,t,:] = S0[k,t] + relu(w)*Up[k,t] + relu(-w)*Un[k,t] + off[b,k]
with Up[k,t] = Pext[k,t-1]-P[k,t], Un[k,t] = Pext[k,t+1]-P[k,t] (edge-replicated,
boundary rows zeroed), valid whenever |warp| < 1 (actual range ~[-0.36, 0.40]).
The warp/off scalars come from the small conv encoder, computed on-device.

Sharding: pure data-parallel over batch (8 rows per core, 8 cores).
Per-core layout: prototype tap tiles are stored bf16 as [128, 4*1280] with
partition p = g*16 + th, free f = q*1280 + tl*16 + c, where k = 8*q + g and
t = th*80 + tl.  This makes every per-(b,q) pass a single [128, 1280] op with
per-partition scalars (each partition group of 16 = one k), and the output DMA
a fully contiguous 640KB store.
"""
import sys

for p in ("/opt/trn_rl_repo",):
    if p not in sys.path:
        sys.path.insert(0, p)

import numpy as np
import ml_dtypes

import concourse.bass as bass
import concourse.mybir as mybir
from concourse import tile, bacc, bass_utils

BF16 = ml_dtypes.bfloat16
B, K, T, C, E = 64, 32, 1280, 16, 64
CONV_CH, POOL = 32, 10
NCORES = 8
BL = B // NCORES        # 8 batch rows per core
NQ = K // 8             # 4 k-groups
TH, TL = 4, 320         # t = th*320 + tl ; partition p = k*4 + th
FD = TL * C             # 5120 free elems per (k, th) row; quarters of 1280
QF = FD // 4

_f32 = mybir.dt.float32
_bf16 = mybir.dt.bfloat16
_i32 = mybir.dt.int32
AF = mybir.ActivationFunctionType
OP = mybir.AluOpType


def _tap_layout(a):
    """[K, T, C] -> [128, FD]: partition = (k, th), free = (tl, c) — a reshape."""
    return np.ascontiguousarray(a.reshape(128, FD))


def build_nc():
    nc = bacc.Bacc(None, target_bir_lowering=False)

    # ---- DRAM I/O ----
    s0_d = nc.dram_tensor("s0", [128, FD], _bf16, kind="ExternalInput")
    up_d = nc.dram_tensor("up", [128, FD], _bf16, kind="ExternalInput")
    un_d = nc.dram_tensor("un", [128, FD], _bf16, kind="ExternalInput")
    xt_d = nc.dram_tensor("xt", [128, BL * T], _bf16, kind="ExternalInput")
    cw_d = nc.dram_tensor("cw", [128, CONV_CH], _bf16, kind="ExternalInput")
    cb_d = nc.dram_tensor("cb", [CONV_CH, 1], _f32, kind="ExternalInput")
    lw_d = nc.dram_tensor("lw", [96, POOL * E], _f32, kind="ExternalInput")
    cb4_d = nc.dram_tensor("cb4", [96, 1], _f32, kind="ExternalInput")
    lb_d = nc.dram_tensor("lb", [E, 1], _f32, kind="ExternalInput")
    ww_d = nc.dram_tensor("ww", [E, K], _f32, kind="ExternalInput")
    wb_d = nc.dram_tensor("wb", [K, 1], _f32, kind="ExternalInput")
    ow_d = nc.dram_tensor("ow", [E, K], _f32, kind="ExternalInput")
    ob_d = nc.dram_tensor("ob", [K, 1], _f32, kind="ExternalInput")

    out_d = nc.dram_tensor("out", [BL, K, T, C], _f32, kind="ExternalOutput")
    warp_d = nc.dram_tensor("warp_o", [BL, K], _f32, kind="ExternalOutput")
    off_d = nc.dram_tensor("off_o", [BL, K], _f32, kind="ExternalOutput")

    with tile.TileContext(nc) as tc:
        with (
            tc.tile_pool(name="taps", bufs=1) as taps,
            tc.tile_pool(name="enc", bufs=1) as enc,
            tc.tile_pool(name="wk", bufs=2) as wk,
            tc.tile_pool(name="outp", bufs=3) as outp,
        ):
            enc_psum = tc.tile_pool(name="pconv", bufs=2, space=bass.MemorySpace.PSUM)
            pconv = enc_psum.__enter__()
            enc_psum2 = tc.tile_pool(name="psmall", bufs=2, space=bass.MemorySpace.PSUM)
            psmall = enc_psum2.__enter__()
            # ---- ACT warmup: force the lazy ACT_TABLE_LOAD at t~0 ----
            wrm = enc.tile([1, 2], _f32)
            nc.gpsimd.memset(wrm[:], 0.0)
            nc.scalar.activation(wrm[:], wrm[:], AF.Relu, bias=0.0, scale=1.0)

            # ---- tiny weights first, then im2col, then taps (FIFO queue) ----
            cw_t = enc.tile([128, CONV_CH], _bf16)
            cb_t = enc.tile([CONV_CH, 1], _f32)
            nc.scalar.dma_start(cw_t[:], cw_d[:])
            nc.scalar.dma_start(cb_t[:], cb_d[:])
            cb3_t = enc.tile([96, 1], _f32)
            nc.scalar.dma_start(cb3_t[:], cb4_d[:])
            lw_t = enc.tile([96, POOL * E], _f32)
            lb_t = enc.tile([E, 1], _f32)
            nc.scalar.dma_start(lw_t[:], lw_d[:])
            nc.scalar.dma_start(lb_t[:], lb_d[:])
            ww_t = enc.tile([E, K], _f32)
            wb_t = enc.tile([K, 1], _f32)
            ow_t = enc.tile([E, K], _f32)
            obb_t = enc.tile([K, 1], _f32)
            nc.scalar.dma_start(ww_t[:], ww_d[:])
            nc.scalar.dma_start(wb_t[:], wb_d[:])
            nc.scalar.dma_start(ow_t[:], ow_d[:])
            nc.scalar.dma_start(obb_t[:], ob_d[:])

            # ---- im2col: one contiguous 80-partition load (host-built) ----
            ic = enc.tile([128, BL * T], _bf16)         # [d*16+c | 48 zero rows]
            nc.sync.dma_start(ic[:], xt_d[:])

            # ---- taps last: up/un first (t1/t2 need them before s0) ----
            s0_t = taps.tile([128, FD], _bf16)
            up_t = taps.tile([128, FD], _bf16)
            un_t = taps.tile([128, FD], _bf16)
            nc.sync.dma_start(up_t[:], up_d[:])
            nc.sync.dma_start(un_t[:], un_d[:])
            nc.sync.dma_start(s0_t[:], s0_d[:])

            # ---- conv: b=3m+n -> partition group 32m, free n*1280 ----
            # (PE matmul out base partition must be 0/32/64 -> 3 groups)
            h3 = enc.tile([96, 3 * T], _bf16)
            pf3 = enc.tile([96, 3 * POOL], _f32)
            for n in range(3):
                nm = 3 if n < 2 else 2           # m-count in this n-column
                ps = pconv.tile([96, T], _f32, tag="ps")
                for m in range(nm):
                    b = 3 * m + n
                    for c0, c1 in ((0, 512), (512, 1024), (1024, T)):
                        nc.tensor.matmul(ps[32 * m:32 * (m + 1), c0:c1], cw_t[:],
                                         ic[:, b * T + c0:b * T + c1],
                                         start=True, stop=True)
                nc.scalar.activation(h3[0:32 * nm, bass.ts(n, T)], ps[0:32 * nm, :],
                                     AF.Relu, bias=cb3_t[0:32 * nm, :], scale=1.0)
                # pool this n-group immediately (overlaps next conv group)
                nc.vector.tensor_reduce(
                    pf3[0:32 * nm, bass.ts(n, POOL)],
                    h3[0:32 * nm, bass.ts(n, T)].rearrange(
                        "p (g i) -> p g i", i=T // POOL),
                    mybir.AxisListType.X, OP.add)

            # ---- linear + relu: per-(m,w) matmuls overlap the reduces ----
            pse = psmall.tile([E, BL], _f32, tag="small")
            pfv = pf3[:].rearrange("p (n w) -> p w n", w=POOL)
            for m in range(3):
                ncols = 3 if m < 2 else 2
                for w in range(POOL):
                    nc.tensor.matmul(pse[:, 3 * m:3 * m + ncols],
                                     lw_t[32 * m:32 * (m + 1), bass.ts(w, E)],
                                     pfv[32 * m:32 * (m + 1), w, 0:ncols],
                                     start=(m == 0 and w == 0),
                                     stop=(m == 2 and w == POOL - 1))
            emb = enc.tile([E, BL], _f32)
            nc.scalar.activation(emb[:], pse[:], AF.Relu,
                                 bias=lb_t[:], scale=1.0 / (T // POOL))
            # ---- heads ----
            psw = psmall.tile([K, BL], _f32, tag="small")
            nc.tensor.matmul(psw[:], ww_t[:], emb[:], start=True, stop=True)
            warp_sb = enc.tile([K, BL], _f32)
            nc.vector.tensor_scalar(warp_sb[:], psw[:], wb_t[:], None, OP.add)

            pso = psmall.tile([K, BL], _f32, tag="small")
            nc.tensor.matmul(pso[:], ow_t[:], emb[:], start=True, stop=True)
            off_sb = enc.tile([K, BL], _f32)
            nc.vector.tensor_scalar(off_sb[:], pso[:], obb_t[:], None, OP.add)

            with nc.allow_non_contiguous_dma(reason="tiny [K,BL] transposed store"):
                nc.gpsimd.dma_start(warp_d[:].rearrange("b k -> k b"), warp_sb[:])
                nc.gpsimd.dma_start(off_d[:].rearrange("b k -> k b"), off_sb[:])

            # ---- per-pair scalars: [wp | wn | off] as [K, 3*BL] ----
            scal = enc.tile([K, 3 * BL], _f32)
            nc.vector.tensor_scalar(scal[:, 0:BL], warp_sb[:], 0.0, None, OP.max)
            nc.vector.tensor_scalar(scal[:, BL:2 * BL], warp_sb[:], -1.0, 0.0,
                                    OP.mult, OP.max)
            nc.vector.tensor_copy(scal[:, 2 * BL:3 * BL], off_sb[:])

            # ---- replicate to [128, 3*BL] via one-hot matmul (p//4 == k) ----
            e2 = enc.tile([K, 128], _i32)
            nc.gpsimd.iota(e2[:], pattern=[[1, 32], [0, 4]], base=0,
                           channel_multiplier=-1)
            rq = enc.tile([K, 128], _f32)
            nc.vector.tensor_scalar(rq[:], e2[:], 0, None, OP.is_equal)
            pss = psmall.tile([128, 3 * BL], _f32, tag="small")
            nc.tensor.matmul(pss[:], rq[:], scal[:], start=True, stop=True)
            scalq = enc.tile([128, 3 * BL], _f32)
            nc.vector.tensor_copy(scalq[:], pss[:])

            enc_psum2.__exit__(None, None, None)
            enc_psum.__exit__(None, None, None)
            pmain_cm = tc.tile_pool(name="pmain", bufs=2, space=bass.MemorySpace.PSUM)
            pmain = pmain_cm.__enter__()

            # ---- bf16 identity for PE tap-joins ----
            ei = enc.tile([128, 128], _i32)
            nc.gpsimd.iota(ei[:], pattern=[[-1, 128]], base=0,
                           channel_multiplier=1)          # q - p
            ident = taps.tile([128, 128], _bf16)
            nc.vector.tensor_scalar(ident[:], ei[:], 0, None, OP.is_equal)

            # ---- main: per-b [128, 5120] DVE; quarter psum+evac; half DMAs ----
            for b in range(BL):
                wp = scalq[:, b:b + 1]
                wn = scalq[:, BL + b:BL + b + 1]
                of = scalq[:, 2 * BL + b:2 * BL + b + 1]

                t1 = wk.tile([128, FD], _bf16, tag="t1")
                nc.vector.tensor_scalar(t1[:], up_t[:], wp, None, OP.mult)
                t2 = wk.tile([128, FD], _bf16, tag="t2")
                nc.vector.tensor_scalar(t2[:], un_t[:], wn, None, OP.mult)
                u = wk.tile([128, FD], _bf16, tag="u")

                ofb = outp.tile([128, FD], _f32, tag="ofb")
                for q4 in range(4):
                    qs = bass.ts(q4, QF)
                    nc.vector.tensor_tensor(u[:, qs], t1[:, qs], t2[:, qs],
                                            OP.add)
                    ps = pmain.tile([128, QF], _f32, tag="ps")
                    for c0, c1 in ((0, 512), (512, 1024), (1024, QF)):
                        s = q4 * QF
                        nc.tensor.matmul(ps[:, c0:c1], ident[:],
                                         u[:, s + c0:s + c1],
                                         start=True, stop=False)
                        nc.tensor.matmul(ps[:, c0:c1], ident[:],
                                         s0_t[:, s + c0:s + c1],
                                         start=False, stop=True)
                    nc.scalar.activation(ofb[:, bass.ts(q4, QF)], ps[:],
                                         AF.Identity, bias=of, scale=1.0)
                    if q4 % 2 == 1:
                        h = q4 // 2
                        dst = out_d[b].rearrange(
                            "k (th tl) c -> (k th) (tl c)", th=TH)
                        nc.sync.dma_start(
                            dst[:, bass.ts(h, FD // 2)],
                            ofb[:, bass.ts(h, FD // 2)])
            pmain_cm.__exit__(None, None, None)

    nc.compile()
    return nc


def host_prep(inputs):
    """Build the per-core in_maps from the full reference inputs (numpy only)."""
    x = np.asarray(inputs["x"], np.float32)
    P = np.asarray(inputs["prototypes"], np.float32)
    conv_w = np.asarray(inputs["conv_w"], np.float32)
    conv_b = np.asarray(inputs["conv_b"], np.float32)
    lin_w = np.asarray(inputs["lin_w"], np.float32)
    lin_b = np.asarray(inputs["lin_b"], np.float32)
    warp_W = np.asarray(inputs["warp_W"], np.float32)
    warp_b = np.asarray(inputs["warp_b"], np.float32)
    off_W = np.asarray(inputs["off_W"], np.float32)
    off_b = np.asarray(inputs["off_b"], np.float32)

    # taps (weight-only prep): S0, Up, Un in tap layout, bf16
    Pm1 = np.concatenate([P[:, :1], P[:, :-1]], axis=1)
    Pp1 = np.concatenate([P[:, 1:], P[:, -1:]], axis=1)
    s0 = _tap_layout(P.astype(BF16))
    up = _tap_layout((Pm1 - P).astype(BF16))
    un = _tap_layout((Pp1 - P).astype(BF16))

    cw = np.zeros((128, CONV_CH), BF16)
    cw[:80] = np.ascontiguousarray(
        conv_w.transpose(2, 1, 0).reshape(80, CONV_CH)).astype(BF16)  # [d*16+c, o]
    cb = conv_b.reshape(CONV_CH, 1).copy()
    lw1 = np.ascontiguousarray(
        lin_w.reshape(E, CONV_CH, POOL).transpose(1, 2, 0).reshape(
            CONV_CH, POOL * E))  # [o, win*64+e]
    lw = np.tile(lw1, (3, 1))
    cb4 = np.tile(conv_b.reshape(CONV_CH, 1), (3, 1))
    lb = lin_b.reshape(E, 1).copy()
    ww = np.ascontiguousarray(warp_W.T)
    wb = warp_b.reshape(K, 1).copy()
    ow = np.ascontiguousarray(off_W.T)
    ob = off_b.reshape(K, 1).copy()

    shared = dict(s0=s0, up=up, un=un, cw=cw, cb=cb, cb4=cb4, lw=lw, lb=lb,
                  ww=ww, wb=wb, ow=ow, ob=ob)
    xt = np.ascontiguousarray(x.transpose(0, 2, 1)).astype(BF16)  # [B, C, T]
    xt_pad = np.zeros((B, C, T + 4), BF16)
    xt_pad[:, :, 2:T + 2] = xt
    in_maps = []
    for i in range(NCORES):
        m = dict(shared)
        xtp = xt_pad[i * BL:(i + 1) * BL]              # [BL, C, T+4]
        icol = np.stack([xtp[:, :, d:d + T] for d in range(5)], axis=0)
        icp = np.zeros((128, BL * T), BF16)
        icp[:80] = icol.transpose(0, 2, 1, 3).reshape(80, BL * T)  # [d*16+c, b*t]
        m["xt"] = icp
        in_maps.append(m)
    return in_maps


_NC_CACHE = {}


def kernel(**inputs):
    if "nc" not in _NC_CACHE:
        _NC_CACHE["nc"] = build_nc()
    nc = _NC_CACHE["nc"]
    in_maps = host_prep(inputs)
    res = bass_utils.run_bass_kernel_spmd(nc, in_maps, core_ids=list(range(NCORES)))
    outs = res.results
    warped = np.concatenate([np.asarray(o["out"]) for o in outs], axis=0)
    warp = np.concatenate([np.asarray(o["warp_o"]) for o in outs], axis=0)
    off = np.concatenate([np.asarray(o["off_o"]) for o in outs], axis=0)
    return warped, warp[..., None], off[..., None]
